# revision 37
# baseline (speedup 1.0000x reference)
"""Distributed attention kernel for 8 trn2 NeuronCores.

Reference semantics (B=2, S=2048, D=2048, H=16, dh=128):
  q = x@W_q, k = x@W_k  (per-head split), v = x@W_v (full width)
  scores = q@k^T per head; (scores + triu(-1e9)) * 1/sqrt(dh); softmax
  out = (sum_h probs_h) @ v @ W_o        <- heads summed, v full width

Sharding: 2 groups of 4 cores (batch parallel); within a group, rank r
owns heads {4r..4r+3} (cols of W_q/W_k), cols [512r, 512r+512) of W_v.
Each core computes P_local = sum of its 4 heads' probs; per-slab
causally-trimmed ReduceScatter(add) over the group sums heads and
shards q rows: rank r gets q-tiles {r, 4+r, 8+r, 12+r}. v is
AllGathered (bf16). Each core then computes Y = (P_own @ v) @ W_o for
its 512 q rows -> no second collective; host concatenates.

Schedule: v projection FIRST so the AllGather overlaps the q/k
projections on the single CC stream; P-matmuls software-pipelined one
q-tile behind the scores so the tensor engine never waits on softmax;
P transposed post-RS on the tensor engine (no DMA transposes); OT/Y
split hi/lo halves so tensor work covers the RS tail.

Precision: score path (x@Wq, x@Wk, q@k^T) in float32r (full PE rate at
free-size>=256); softmax in f32; P through RS in bf16; v/P^T/O/W_o in
bf16 with f32 PSUM accumulation.
"""

import math

import numpy as np
import ml_dtypes

import concourse.bass as bass
import concourse.mybir as mybir
import concourse.tile as tile
from concourse import bacc
from concourse.bass_utils import run_bass_kernel_spmd
from concourse.masks import make_identity

F32 = mybir.dt.float32
F32R = mybir.dt.float32r
BF16 = mybir.dt.bfloat16

S = 2048
D = 2048
DH = 128
NT = S // 128  # 16 q/k tiles
SCALE = 1.0 / math.sqrt(DH)
GROUPS = [[0, 1, 2, 3], [4, 5, 6, 7]]
NEG = -1e9


def build():
    nc = bacc.Bacc("TRN2", target_bir_lowering=False, debug=False, num_devices=8)

    x = nc.declare_dram_parameter("x", [D, S], F32R, isOutput=False)  # x TRANSPOSED on host
    xbf = nc.declare_dram_parameter("xbf", [D, S], BF16, isOutput=False)
    wq = nc.declare_dram_parameter("wq", [D, 512], F32R, isOutput=False)
    wk = nc.declare_dram_parameter("wk", [D, 512], F32R, isOutput=False)
    wv = nc.declare_dram_parameter("wv", [D, 512], BF16, isOutput=False)
    wo = nc.declare_dram_parameter("wo", [D, D], BF16, isOutput=False)
    out = nc.declare_dram_parameter("out", [512, D], F32, isOutput=True)

    # causally trimmed P slabs: slab s covers q rows [512s, 512s+512),
    # keys [0, (s+1)*512)
    p_dram = [nc.dram_tensor(f"p_dram{s}", [512, (s + 1) * 512], BF16) for s in range(4)]
    p_own = [nc.dram_tensor(f"p_own{s}", [128, (s + 1) * 512], BF16) for s in range(4)]
    # v stored partition-major [128, kt, dv] so post-AG loads are contiguous
    v_local = nc.dram_tensor("v_local", [128, NT, 512], BF16)
    v_ag = nc.dram_tensor("v_ag", [4, 128, NT, 512], BF16)

    xbf_r = xbf.rearrange("(t p) s -> p t s", p=128)
    x_r = x.rearrange("(t p) s -> p t s", p=128)
    wq_r = wq.rearrange("(t p) c -> p t c", p=128)
    wk_r = wk.rearrange("(t p) c -> p t c", p=128)
    wv_r = wv.rearrange("(t p) c -> p t c", p=128)
    wo_r = wo.rearrange("(t p) c -> p t c", p=128)

    with tile.TileContext(nc) as tc:
        with tc.tile_pool(name="const", bufs=1) as cst:
            ident = cst.tile([128, 128], F32)
            make_identity(nc, ident)
            ident_bf = cst.tile([128, 128], BF16)
            nc.vector.tensor_copy(out=ident_bf[:], in_=ident[:])
            # triangular mask for the diagonal 128x128 block:
            # 0 where col <= row else -1e9
            zero512 = cst.tile([128, 512], BF16)
            nc.vector.memset(zero512[:], 0.0)
            mask = cst.tile([128, 128], BF16)
            nc.gpsimd.memset(mask[:], 0.0)
            nc.gpsimd.affine_select(
                out=mask[:],
                in_=mask[:],
                compare_op=mybir.AluOpType.is_ge,
                fill=NEG,
                base=0,
                pattern=[[-1, 128]],
                channel_multiplier=1,
            )

            with tc.tile_pool(name="qk", bufs=1) as qkp:
                qT = qkp.tile([128, 4, S], F32R)  # [dh-part, head, q]
                kT = qkp.tile([128, 4, S], F32R)

                with (
                    tc.tile_pool(name="wqk", bufs=1) as wqkp,
                    tc.tile_pool(name="proj_ps", bufs=8, space="PSUM") as pjp,
                ):
                    wq_sb = wqkp.tile([128, NT, 512], F32R)
                    wk_sb = wqkp.tile([128, NT, 512], F32R)

                    # ------------ Phase B: v = x @ wv, then AllGather ------
                    with (
                        tc.tile_pool(name="bpool", bufs=1) as bp,
                        tc.tile_pool(name="xbq", bufs=24) as xbp,
                        tc.tile_pool(name="vsb", bufs=8) as vsbp,
                    ):
                        wv_t = bp.tile([128, NT, 512], BF16)
                        nc.sync.dma_start(wv_t[:, 0:8, :], wv_r[:, 0:8, :])
                        nc.scalar.dma_start(wv_t[:, 8:16, :], wv_r[:, 8:16, :])
                        for qtr in range(4):
                            s0 = qtr * 512
                            psums = [
                                pjp.tile([128, 512], F32, tag="ps512", name=f"pv{qtr}_{j}")
                                for j in range(4)
                            ]
                            for Dt in range(NT):
                                xb_t = xbp.tile([128, 512], BF16, tag="xb")
                                q_eng = nc.sync if Dt % 2 == 0 else nc.scalar
                                q_eng.dma_start(
                                    xb_t[:], xbf_r[:, Dt, s0 : s0 + 512]
                                )
                                for j in range(4):
                                    nc.tensor.matmul(
                                        psums[j][:],
                                        xb_t[:, j * 128 : (j + 1) * 128],
                                        wv_t[:, Dt, :],
                                        start=(Dt == 0),
                                        stop=(Dt == NT - 1),
                                    )
                            for j in range(4):
                                kt = qtr * 4 + j
                                v_sb = vsbp.tile([128, 512], BF16, tag="vsb")
                                if j % 2 == 0:
                                    nc.vector.tensor_copy(out=v_sb[:], in_=psums[j][:])
                                else:
                                    nc.scalar.copy(out=v_sb[:], in_=psums[j][:])
                                nc.scalar.dma_start(v_local[:, kt, :], v_sb[:])
                            # prefetch wq/wk for phase A during B compute
                            # (gpsimd SWDGE queue: idle, keeps HWDGE queues
                            # free for x loads)
                            if qtr == 0:
                                nc.gpsimd.dma_start(wq_sb[:], wq_r[:])
                            elif qtr == 2:
                                nc.gpsimd.dma_start(wk_sb[:], wk_r[:])
                        nc.gpsimd.collective_compute(
                            "AllGather",
                            mybir.AluOpType.bypass,
                            ins=[v_local[:]],
                            outs=[v_ag[:]],
                            replica_groups=GROUPS,
                        )

                    # ------------ Phase A: q/k projections -----------------
                    with tc.tile_pool(name="xt_pool", bufs=24) as xtp:
                        for qtr in range(4):
                            s0 = qtr * 512
                            xts = []
                            for dst, wsb in ((qT, wq_sb), (kT, wk_sb)):
                                psums = [
                                    pjp.tile(
                                        [128, 512], F32, tag="ps512", name=f"pa{_j}"
                                    )
                                    for _j in range(4)
                                ]
                                for Dt in range(NT):
                                    if dst is qT:
                                        xt = xtp.tile([128, 512], F32R, tag="xt")
                                        q_eng = nc.sync if Dt % 2 == 0 else nc.scalar
                                        q_eng.dma_start(
                                            xt[:], x_r[:, Dt, s0 : s0 + 512]
                                        )
                                        xts.append(xt)
                                    for dt in range(4):
                                        nc.tensor.matmul(
                                            psums[dt][:],
                                            wsb[:, Dt, dt * 128 : (dt + 1) * 128],
                                            xts[Dt][:],
                                            start=(Dt == 0),
                                            stop=(Dt == NT - 1),
                                        )
                                for dt in range(4):
                                    dsl = dst[:, dt, s0 : s0 + 512]
                                    if dt % 2 == 0:
                                        nc.vector.tensor_copy(out=dsl, in_=psums[dt][:])
                                    else:
                                        nc.scalar.copy(out=dsl, in_=psums[dt][:])

                # ------------ Phase C: scores / softmax / P ----------------
                # slabs processed 3,2,1,0 (largest first) so the last RS is
                # the smallest; P-matmuls pipelined one q-tile behind scores
                with (
                    tc.tile_pool(name="epool", bufs=4) as ep,
                    tc.tile_pool(name="small", bufs=48) as smp,
                    tc.tile_pool(name="dsm", bufs=16) as dsm,
                    tc.tile_pool(name="psb", bufs=8) as psbp,
                    tc.tile_pool(name="sc_ps", bufs=3, space="PSUM") as scp,
                    tc.tile_pool(name="p_ps", bufs=2, space="PSUM") as ppp,
                ):
                    i_order = [
                        i for sl in (3, 2, 1, 0) for i in range(4 * sl, 4 * sl + 4)
                    ]
                    slab_done = {0: 0, 1: 0, 2: 0, 3: 0}
                    pending = {}  # i -> (e_t, d_hs)
                    dh_lag = []  # deferred d_h builds (vector, lagged one head)

                    def warm(n=1):
                        # filler matmuls: keep the PE activity monitor from
                        # seeing an idle window (which would halve the clock)
                        for _ in range(n):
                            w_ps = ppp.tile([128, 512], F32, tag="P", name="warm")
                            nc.tensor.matmul(
                                w_ps[:], ident_bf[:], zero512[:],
                                start=True, stop=True,
                            )

                    def flush_dh():
                        while dh_lag:
                            dh_lag.pop(0)()

                    def scores_softmax(i):
                        kwe = (i + 1) * 128  # exact causal width
                        kwc = i // 4 + 1
                        kw = kwc * 512
                        ntile = (kw + 1023) // 1024
                        e_t = ep.tile([128, 4, 2048], BF16, tag="E")
                        warm(1)
                        d_hs = []
                        for h in range(4):
                            if kwe < kw:
                                # zero the masked tail so P chunks self-zero;
                                # on vector: gpsimd blocks inside collectives
                                nc.vector.memset(e_t[:, h, kwe:kw], 0.0)
                            s_tiles = [
                                scp.tile([128, 1024], F32, tag="S", name=f"sch{_j}")
                                for _j in range(ntile)
                            ]
                            for kc in range(kwc):
                                w = 512 if kc < kwc - 1 else max(256, kwe - 512 * kc)
                                nc.tensor.matmul(
                                    s_tiles[kc // 2][
                                        :, (kc % 2) * 512 : (kc % 2) * 512 + w
                                    ],
                                    qT[:, h, i * 128 : (i + 1) * 128],
                                    kT[:, h, kc * 512 : kc * 512 + w],
                                    start=True,
                                    stop=True,
                                )
                            warm(1)
                            # vector block: only tensor-engine dependencies, so
                            # the vector queue never waits on scalar's exp
                            dcol = i * 128
                            dsl = s_tiles[dcol // 1024][
                                :, dcol % 1024 : dcol % 1024 + 128
                            ]
                            nc.vector.tensor_tensor(
                                out=dsl, in0=dsl, in1=mask[:], op=mybir.AluOpType.add
                            )
                            mx = None
                            for t in range(ntile):
                                w = min(kwe - 1024 * t, 1024)
                                if w <= 0:
                                    break
                                mxt = smp.tile([128, 1], F32, tag="mx")
                                nc.vector.reduce_max(
                                    out=mxt[:],
                                    in_=s_tiles[t][:, :w],
                                    axis=mybir.AxisListType.X,
                                )
                                if mx is None:
                                    mx = mxt
                                else:
                                    mxn = smp.tile([128, 1], F32, tag="mx")
                                    nc.vector.tensor_tensor(
                                        out=mxn[:],
                                        in0=mx[:],
                                        in1=mxt[:],
                                        op=mybir.AluOpType.max,
                                    )
                                    mx = mxn
                            nmS = smp.tile([128, 1], F32, tag="mx")
                            nc.vector.tensor_scalar_mul(nmS[:], mx[:], -SCALE)
                            # previous head's d_h: its rinv is ready by now
                            flush_dh()
                            # scalar block: exp + row-sum chain stays on scalar
                            rtot = None
                            for t in range(ntile):
                                w = min(kwe - 1024 * t, 1024)
                                if w <= 0:
                                    break
                                rc = smp.tile([128, 1], F32, tag="mx")
                                nc.scalar.activation(
                                    out=e_t[:, h, 1024 * t : 1024 * t + w],
                                    in_=s_tiles[t][:, :w],
                                    func=mybir.ActivationFunctionType.Exp,
                                    bias=nmS[:],
                                    scale=SCALE,
                                    accum_out=rc[:],
                                )
                                if rtot is None:
                                    rtot = rc
                                else:
                                    rn = smp.tile([128, 1], F32, tag="mx")
                                    nc.scalar.activation(
                                        out=rn[:],
                                        in_=rc[:],
                                        func=mybir.ActivationFunctionType.Identity,
                                        bias=rtot[:],
                                    )
                                    rtot = rn
                            d_h = dsm.tile([128, 128], BF16, tag="D")

                            def build_dh(d_h=d_h, rtot=rtot):
                                rinv = smp.tile([128, 1], F32, tag="mx")
                                nc.vector.reciprocal(out=rinv[:], in_=rtot[:])
                                nc.vector.tensor_scalar_mul(
                                    d_h[:], ident_bf[:], rinv[:]
                                )

                            dh_lag.append(build_dh)
                            d_hs.append(d_h)
                        pending[i] = (e_t, d_hs)

                    def emit_P(i):
                        e_t, d_hs = pending.pop(i)
                        kwc = i // 4 + 1
                        s_idx = i // 4
                        r0 = (i % 4) * 128
                        for kc in range(kwc):
                            p_t = ppp.tile([128, 512], F32, tag="P")
                            for h in range(4):
                                nc.tensor.matmul(
                                    p_t[:],
                                    d_hs[h][:],
                                    e_t[:, h, kc * 512 : (kc + 1) * 512],
                                    start=(h == 0),
                                    stop=(h == 3),
                                )
                            pc = psbp.tile([128, 512], BF16, tag="psb")
                            nc.scalar.copy(out=pc[:], in_=p_t[:])
                            nc.sync.dma_start(
                                p_dram[s_idx][r0 : r0 + 128, kc * 512 : (kc + 1) * 512],
                                pc[:],
                            )
                        slab_done[s_idx] += 1
                        if slab_done[s_idx] == 4:
                            nc.gpsimd.collective_compute(
                                "ReduceScatter",
                                mybir.AluOpType.add,
                                ins=[p_dram[s_idx][:]],
                                outs=[p_own[s_idx][:]],
                                replica_groups=GROUPS,
                            )

                    prev = None
                    for idx, i in enumerate(i_order):
                        if idx == 1:
                            # solid ~4us filler block: promote the PE clock
                            # out of its idle-demoted state early in phase C
                            warm(9)
                        scores_softmax(i)
                        if prev is not None:
                            emit_P(prev)
                        prev = i
                    flush_dh()
                    emit_P(prev)

            # ------------ Phase D: transpose P, OT, Y ----------------------
            with (
                tc.tile_pool(name="pown", bufs=1) as pwp,
                tc.tile_pool(name="dpool", bufs=1) as dp,
                tc.tile_pool(name="ysb", bufs=2) as ysbp,
            ):
                pown_sb = [
                    pwp.tile([128, (s + 1) * 512], BF16, name=f"pown{s}")
                    for s in range(4)
                ]
                nc.sync.dma_start(pown_sb[3][:], p_own[3][:])
                nc.sync.dma_start(pown_sb[2][:], p_own[2][:])
                vf = dp.tile([128, 4, NT, 512], BF16)  # [k-part, g, kt, dv-in-g]
                for g in range(4):
                    q_eng = nc.sync if g % 2 == 0 else nc.scalar
                    q_eng.dma_start(vf[:, g, :, :], v_ag[g][:, :, :])
                wo_sb = dp.tile([128, NT, D], BF16)
                nc.scalar.dma_start(wo_sb[:], wo_r[:])
                pt = dp.tile([128, NT, 512], BF16)  # [k-part, kt, own-q-col]
                ot = dp.tile([128, NT, 512], BF16)  # [dv-part, dvt, own-q]

                with (
                    tc.tile_pool(name="tp_ps", bufs=2, space="PSUM") as tpp,
                    tc.tile_pool(name="ot_ps", bufs=2, space="PSUM") as otbp,
                    tc.tile_pool(name="y_ps", bufs=2, space="PSUM") as yps,
                ):
                    def transpose_slab(s_idx):
                        for kt in range(4 * s_idx + 4):
                            ps = tpp.tile([128, 128], BF16, tag="tp")
                            nc.tensor.transpose(
                                ps[:],
                                pown_sb[s_idx][:, kt * 128 : (kt + 1) * 128],
                                ident_bf[:],
                            )
                            nc.scalar.copy(
                                out=pt[:, kt, s_idx * 128 : (s_idx + 1) * 128],
                                in_=ps[:],
                            )

                    def ot_pass(base, ktn):
                        # OT for own q-cols [base, base+256): k-tiles 0..ktn-1
                        for dvt in range(NT):
                            po = otbp.tile([128, 256], F32, tag="OTB")
                            for kt in range(ktn):
                                c0 = max(0, 128 * (kt // 4) - base)
                                nc.tensor.matmul(
                                    po[:, c0:256],
                                    vf[
                                        :,
                                        dvt // 4,
                                        kt,
                                        (dvt % 4) * 128 : (dvt % 4) * 128 + 128,
                                    ],
                                    pt[:, kt, base + c0 : base + 256],
                                    start=(kt == 0),
                                    stop=(kt == ktn - 1),
                                )
                            nc.vector.tensor_copy(
                                out=ot[:, dvt, base : base + 256], in_=po[:]
                            )

                    def y_pass(qb):
                        for nch in range(4):
                            yp = yps.tile([128, 512], F32, tag="Y")
                            for dvt in range(NT):
                                nc.tensor.matmul(
                                    yp[:],
                                    ot[:, dvt, qb * 128 : (qb + 1) * 128],
                                    wo_sb[:, dvt, nch * 512 : nch * 512 + 512],
                                    start=(dvt == 0),
                                    stop=(dvt == NT - 1),
                                )
                            y_sb = ysbp.tile([128, 512], F32, tag="ysb")
                            nc.scalar.copy(out=y_sb[:], in_=yp[:])
                            nc.sync.dma_start(
                                out[
                                    qb * 128 : (qb + 1) * 128,
                                    nch * 512 : nch * 512 + 512,
                                ],
                                y_sb[:],
                            )

                    transpose_slab(3)
                    transpose_slab(2)
                    ot_pass(256, 16)
                    y_pass(3)
                    y_pass(2)
                    nc.sync.dma_start(pown_sb[1][:], p_own[1][:])
                    nc.sync.dma_start(pown_sb[0][:], p_own[0][:])
                    transpose_slab(1)
                    transpose_slab(0)
                    ot_pass(0, 8)
                    y_pass(1)
                    y_pass(0)

    nc.compile()
    return nc


_NC_CACHE = None


def kernel(x, W_q, W_k, W_v, W_o):
    global _NC_CACHE
    x = np.asarray(x, dtype=np.float32)
    W_q = np.asarray(W_q, dtype=np.float32)
    W_k = np.asarray(W_k, dtype=np.float32)
    W_v = np.asarray(W_v, dtype=np.float32)
    W_o = np.asarray(W_o, dtype=np.float32)
    if _NC_CACHE is None:
        _NC_CACHE = build()
    nc = _NC_CACHE

    wo_bf = W_o.astype(ml_dtypes.bfloat16)
    xT = [np.ascontiguousarray(x[g].T) for g in range(2)]
    xT_bf = [t.astype(ml_dtypes.bfloat16) for t in xT]
    in_maps = []
    for c in range(8):
        g, r = divmod(c, 4)
        in_maps.append(
            {
                "x": xT[g],
                "xbf": xT_bf[g],
                "wq": np.ascontiguousarray(W_q[:, 512 * r : 512 * (r + 1)]),
                "wk": np.ascontiguousarray(W_k[:, 512 * r : 512 * (r + 1)]),
                "wv": np.ascontiguousarray(W_v[:, 512 * r : 512 * (r + 1)]).astype(ml_dtypes.bfloat16),
                "wo": wo_bf,
            }
        )
    res = run_bass_kernel_spmd(nc, in_maps, core_ids=list(range(8)))
    Y = np.empty((2, S, D), dtype=np.float32)
    for c in range(8):
        g, r = divmod(c, 4)
        o = res.results[c]["out"]
        for s_idx in range(4):
            t = 4 * s_idx + r
            Y[g, t * 128 : (t + 1) * 128, :] = o[s_idx * 128 : (s_idx + 1) * 128, :]
    return Y


# revision 39
# speedup vs baseline: 1.0159x; 1.0159x over previous
"""Distributed attention kernel for 8 trn2 NeuronCores.

Reference semantics (B=2, S=2048, D=2048, H=16, dh=128):
  q = x@W_q, k = x@W_k  (per-head split), v = x@W_v (full width)
  scores = q@k^T per head; (scores + triu(-1e9)) * 1/sqrt(dh); softmax
  out = (sum_h probs_h) @ v @ W_o        <- heads summed, v full width

Sharding: 2 groups of 4 cores (batch parallel); within a group, rank r
owns heads {4r..4r+3} (cols of W_q/W_k), cols [512r, 512r+512) of W_v.
Each core computes P_local = sum of its 4 heads' probs; per-slab
causally-trimmed ReduceScatter(add) over the group sums heads and
shards q rows: rank r gets q-tiles {r, 4+r, 8+r, 12+r}. v is
AllGathered (bf16). Each core then computes Y = (P_own @ v) @ W_o for
its 512 q rows -> no second collective; host concatenates.

Schedule: v projection FIRST so the AllGather overlaps the q/k
projections on the single CC stream; P-matmuls software-pipelined one
q-tile behind the scores so the tensor engine never waits on softmax;
P transposed post-RS on the tensor engine (no DMA transposes); OT/Y
split hi/lo halves so tensor work covers the RS tail.

Precision: score path (x@Wq, x@Wk, q@k^T) in float32r (full PE rate at
free-size>=256); softmax in f32; P through RS in bf16; v/P^T/O/W_o in
bf16 with f32 PSUM accumulation.
"""

import math

import numpy as np
import ml_dtypes

import concourse.bass as bass
import concourse.mybir as mybir
import concourse.tile as tile
from concourse import bacc
from concourse.bass_utils import run_bass_kernel_spmd
from concourse.masks import make_identity

F32 = mybir.dt.float32
F32R = mybir.dt.float32r
BF16 = mybir.dt.bfloat16

S = 2048
D = 2048
DH = 128
NT = S // 128  # 16 q/k tiles
SCALE = 1.0 / math.sqrt(DH)
GROUPS = [[0, 1, 2, 3], [4, 5, 6, 7]]
NEG = -1e9


def build():
    nc = bacc.Bacc("TRN2", target_bir_lowering=False, debug=False, num_devices=8)

    x = nc.declare_dram_parameter("x", [D, S], F32R, isOutput=False)  # x TRANSPOSED on host
    xbf = nc.declare_dram_parameter("xbf", [D, S], BF16, isOutput=False)
    wq = nc.declare_dram_parameter("wq", [D, 512], F32R, isOutput=False)
    wk = nc.declare_dram_parameter("wk", [D, 512], F32R, isOutput=False)
    wv = nc.declare_dram_parameter("wv", [D, 512], BF16, isOutput=False)
    wo = nc.declare_dram_parameter("wo", [D, D], BF16, isOutput=False)
    out = nc.declare_dram_parameter("out", [512, D], F32, isOutput=True)

    # causally trimmed P slabs: slab s covers q rows [512s, 512s+512),
    # keys [0, (s+1)*512)
    p_dram = [nc.dram_tensor(f"p_dram{s}", [512, (s + 1) * 512], BF16) for s in range(4)]
    p_own = [nc.dram_tensor(f"p_own{s}", [128, (s + 1) * 512], BF16) for s in range(4)]
    # v stored partition-major [128, kt, dv] so post-AG loads are contiguous
    v_local = nc.dram_tensor("v_local", [128, NT, 512], BF16)
    v_ag = nc.dram_tensor("v_ag", [4, 128, NT, 512], BF16)

    xbf_r = xbf.rearrange("(t p) s -> p t s", p=128)
    x_r = x.rearrange("(t p) s -> p t s", p=128)
    wq_r = wq.rearrange("(t p) c -> p t c", p=128)
    wk_r = wk.rearrange("(t p) c -> p t c", p=128)
    wv_r = wv.rearrange("(t p) c -> p t c", p=128)
    wo_r = wo.rearrange("(t p) c -> p t c", p=128)

    with tile.TileContext(nc) as tc:
        with tc.tile_pool(name="const", bufs=1) as cst:
            ident = cst.tile([128, 128], F32)
            make_identity(nc, ident)
            ident_bf = cst.tile([128, 128], BF16)
            nc.vector.tensor_copy(out=ident_bf[:], in_=ident[:])
            # triangular mask for the diagonal 128x128 block:
            # 0 where col <= row else -1e9
            zero512 = cst.tile([128, 512], BF16)
            nc.vector.memset(zero512[:], 0.0)
            mask = cst.tile([128, 128], BF16)
            nc.gpsimd.memset(mask[:], 0.0)
            nc.gpsimd.affine_select(
                out=mask[:],
                in_=mask[:],
                compare_op=mybir.AluOpType.is_ge,
                fill=NEG,
                base=0,
                pattern=[[-1, 128]],
                channel_multiplier=1,
            )

            with tc.tile_pool(name="qk", bufs=1) as qkp:
                qT = qkp.tile([128, 4, S], F32R)  # [dh-part, head, q]
                kT = qkp.tile([128, 4, S], F32R)

                with (
                    tc.tile_pool(name="wqk", bufs=1) as wqkp,
                    tc.tile_pool(name="proj_ps", bufs=8, space="PSUM") as pjp,
                ):
                    wq_sb = wqkp.tile([128, NT, 512], F32R)
                    wk_sb = wqkp.tile([128, NT, 512], F32R)

                    # ------------ Phase B: v = x @ wv, then AllGather ------
                    with (
                        tc.tile_pool(name="bpool", bufs=1) as bp,
                        tc.tile_pool(name="xbq", bufs=24) as xbp,
                        tc.tile_pool(name="vsb", bufs=8) as vsbp,
                    ):
                        wv_t = bp.tile([128, NT, 512], BF16)
                        nc.sync.dma_start(wv_t[:, 0:8, :], wv_r[:, 0:8, :])
                        nc.scalar.dma_start(wv_t[:, 8:16, :], wv_r[:, 8:16, :])
                        for qtr in range(4):
                            s0 = qtr * 512
                            psums = [
                                pjp.tile([128, 512], F32, tag="ps512", name=f"pv{qtr}_{j}")
                                for j in range(4)
                            ]
                            for Dt in range(NT):
                                xb_t = xbp.tile([128, 512], BF16, tag="xb")
                                q_eng = nc.sync if Dt % 2 == 0 else nc.scalar
                                q_eng.dma_start(
                                    xb_t[:], xbf_r[:, Dt, s0 : s0 + 512]
                                )
                                for j in range(4):
                                    nc.tensor.matmul(
                                        psums[j][:],
                                        xb_t[:, j * 128 : (j + 1) * 128],
                                        wv_t[:, Dt, :],
                                        start=(Dt == 0),
                                        stop=(Dt == NT - 1),
                                    )
                            for j in range(4):
                                kt = qtr * 4 + j
                                v_sb = vsbp.tile([128, 512], BF16, tag="vsb")
                                if j % 2 == 0:
                                    nc.vector.tensor_copy(out=v_sb[:], in_=psums[j][:])
                                else:
                                    nc.scalar.copy(out=v_sb[:], in_=psums[j][:])
                                nc.scalar.dma_start(v_local[:, kt, :], v_sb[:])
                            # prefetch wq/wk for phase A during B compute
                            # (gpsimd SWDGE queue: idle, keeps HWDGE queues
                            # free for x loads)
                            if qtr == 0:
                                nc.gpsimd.dma_start(wq_sb[:], wq_r[:])
                            elif qtr == 2:
                                nc.gpsimd.dma_start(wk_sb[:], wk_r[:])
                        nc.gpsimd.collective_compute(
                            "AllGather",
                            mybir.AluOpType.bypass,
                            ins=[v_local[:]],
                            outs=[v_ag[:]],
                            replica_groups=GROUPS,
                        )

                    # ------------ Phase A: q/k projections -----------------
                    with tc.tile_pool(name="xt_pool", bufs=24) as xtp:
                        for qtr in range(4):
                            s0 = qtr * 512
                            xts = []
                            for dst, wsb in ((qT, wq_sb), (kT, wk_sb)):
                                psums = [
                                    pjp.tile(
                                        [128, 512], F32, tag="ps512", name=f"pa{_j}"
                                    )
                                    for _j in range(4)
                                ]
                                for Dt in range(NT):
                                    if dst is qT:
                                        xt = xtp.tile([128, 512], F32R, tag="xt")
                                        q_eng = nc.sync if Dt % 2 == 0 else nc.scalar
                                        q_eng.dma_start(
                                            xt[:], x_r[:, Dt, s0 : s0 + 512]
                                        )
                                        xts.append(xt)
                                    for dt in range(4):
                                        nc.tensor.matmul(
                                            psums[dt][:],
                                            wsb[:, Dt, dt * 128 : (dt + 1) * 128],
                                            xts[Dt][:],
                                            start=(Dt == 0),
                                            stop=(Dt == NT - 1),
                                        )
                                for dt in range(4):
                                    dsl = dst[:, dt, s0 : s0 + 512]
                                    if dt % 2 == 0:
                                        nc.vector.tensor_copy(out=dsl, in_=psums[dt][:])
                                    else:
                                        nc.scalar.copy(out=dsl, in_=psums[dt][:])

                # ------------ Phase C: scores / softmax / P ----------------
                # slabs processed 3,2,1,0 (largest first) so the last RS is
                # the smallest; P-matmuls pipelined one q-tile behind scores
                with (
                    tc.tile_pool(name="epool", bufs=3) as ep,
                    tc.tile_pool(name="small", bufs=32) as smp,
                    tc.tile_pool(name="dsm", bufs=12) as dsm,
                    tc.tile_pool(name="psb", bufs=6) as psbp,
                    tc.tile_pool(name="sc_ps", bufs=3, space="PSUM") as scp,
                    tc.tile_pool(name="p_ps", bufs=2, space="PSUM") as ppp,
                ):
                    i_order = [
                        i for sl in (3, 2, 1, 0) for i in range(4 * sl, 4 * sl + 4)
                    ]
                    slab_done = {0: 0, 1: 0, 2: 0, 3: 0}
                    pending = {}  # i -> (e_t, d_hs)
                    dh_lag = []  # deferred d_h builds (vector, lagged one head)

                    def warm(n=1):
                        # filler matmuls: keep the PE activity monitor from
                        # seeing an idle window (which would halve the clock)
                        for _ in range(n):
                            w_ps = ppp.tile([128, 512], F32, tag="P", name="warm")
                            nc.tensor.matmul(
                                w_ps[:], ident_bf[:], zero512[:],
                                start=True, stop=True,
                            )

                    def flush_dh():
                        while dh_lag:
                            dh_lag.pop(0)()

                    def scores_softmax(i):
                        kwe = (i + 1) * 128  # exact causal width
                        kwc = i // 4 + 1
                        kw = kwc * 512
                        ntile = (kw + 1023) // 1024
                        e_t = ep.tile([128, 4, 2048], BF16, tag="E")
                        d_hs = []
                        for h in range(4):
                            if kwe < kw:
                                # zero the masked tail so P chunks self-zero;
                                # on vector: gpsimd blocks inside collectives
                                nc.vector.memset(e_t[:, h, kwe:kw], 0.0)
                            s_tiles = [
                                scp.tile([128, 1024], F32, tag="S", name=f"sch{_j}")
                                for _j in range(ntile)
                            ]
                            for kc in range(kwc):
                                w = 512 if kc < kwc - 1 else max(256, kwe - 512 * kc)
                                nc.tensor.matmul(
                                    s_tiles[kc // 2][
                                        :, (kc % 2) * 512 : (kc % 2) * 512 + w
                                    ],
                                    qT[:, h, i * 128 : (i + 1) * 128],
                                    kT[:, h, kc * 512 : kc * 512 + w],
                                    start=True,
                                    stop=True,
                                )
                            warm(1)
                            # vector block: only tensor-engine dependencies, so
                            # the vector queue never waits on scalar's exp
                            dcol = i * 128
                            dsl = s_tiles[dcol // 1024][
                                :, dcol % 1024 : dcol % 1024 + 128
                            ]
                            nc.vector.tensor_tensor(
                                out=dsl, in0=dsl, in1=mask[:], op=mybir.AluOpType.add
                            )
                            mx = None
                            for t in range(ntile):
                                w = min(kwe - 1024 * t, 1024)
                                if w <= 0:
                                    break
                                mxt = smp.tile([128, 1], F32, tag="mx")
                                nc.vector.reduce_max(
                                    out=mxt[:],
                                    in_=s_tiles[t][:, :w],
                                    axis=mybir.AxisListType.X,
                                )
                                if mx is None:
                                    mx = mxt
                                else:
                                    mxn = smp.tile([128, 1], F32, tag="mx")
                                    nc.vector.tensor_tensor(
                                        out=mxn[:],
                                        in0=mx[:],
                                        in1=mxt[:],
                                        op=mybir.AluOpType.max,
                                    )
                                    mx = mxn
                            nmS = smp.tile([128, 1], F32, tag="mx")
                            nc.vector.tensor_scalar_mul(nmS[:], mx[:], -SCALE)
                            # previous head's d_h: its rinv is ready by now
                            flush_dh()
                            # scalar block: exp + row-sum chain stays on scalar
                            rtot = None
                            for t in range(ntile):
                                w = min(kwe - 1024 * t, 1024)
                                if w <= 0:
                                    break
                                rc = smp.tile([128, 1], F32, tag="mx")
                                nc.scalar.activation(
                                    out=e_t[:, h, 1024 * t : 1024 * t + w],
                                    in_=s_tiles[t][:, :w],
                                    func=mybir.ActivationFunctionType.Exp,
                                    bias=nmS[:],
                                    scale=SCALE,
                                    accum_out=rc[:],
                                )
                                if rtot is None:
                                    rtot = rc
                                else:
                                    rn = smp.tile([128, 1], F32, tag="mx")
                                    nc.scalar.activation(
                                        out=rn[:],
                                        in_=rc[:],
                                        func=mybir.ActivationFunctionType.Identity,
                                        bias=rtot[:],
                                    )
                                    rtot = rn
                            d_h = dsm.tile([128, 128], BF16, tag="D")

                            def build_dh(d_h=d_h, rtot=rtot):
                                rinv = smp.tile([128, 1], F32, tag="mx")
                                nc.vector.reciprocal(out=rinv[:], in_=rtot[:])
                                nc.vector.tensor_scalar_mul(
                                    d_h[:], ident_bf[:], rinv[:]
                                )

                            dh_lag.append(build_dh)
                            d_hs.append(d_h)
                        pending[i] = (e_t, d_hs)

                    def emit_P(i):
                        e_t, d_hs = pending.pop(i)
                        kwc = i // 4 + 1
                        s_idx = i // 4
                        r0 = (i % 4) * 128
                        for kc in range(kwc):
                            p_t = ppp.tile([128, 512], F32, tag="P")
                            for h in range(4):
                                nc.tensor.matmul(
                                    p_t[:],
                                    d_hs[h][:],
                                    e_t[:, h, kc * 512 : (kc + 1) * 512],
                                    start=(h == 0),
                                    stop=(h == 3),
                                )
                            pc = psbp.tile([128, 512], BF16, tag="psb")
                            nc.scalar.copy(out=pc[:], in_=p_t[:])
                            nc.sync.dma_start(
                                p_dram[s_idx][r0 : r0 + 128, kc * 512 : (kc + 1) * 512],
                                pc[:],
                            )
                        slab_done[s_idx] += 1
                        if slab_done[s_idx] == 4:
                            nc.gpsimd.collective_compute(
                                "ReduceScatter",
                                mybir.AluOpType.add,
                                ins=[p_dram[s_idx][:]],
                                outs=[p_own[s_idx][:]],
                                replica_groups=GROUPS,
                            )

                    prev = None
                    for idx, i in enumerate(i_order):
                        if idx == 1:
                            # solid ~4us filler block: promote the PE clock
                            # out of its idle-demoted state early in phase C
                            warm(9)
                        scores_softmax(i)
                        if prev is not None:
                            emit_P(prev)
                        prev = i
                    flush_dh()
                    emit_P(prev)

            # ------------ Phase D: transpose P, OT, Y ----------------------
            with (
                tc.tile_pool(name="pown", bufs=1) as pwp,
                tc.tile_pool(name="dpool", bufs=1) as dp,
                tc.tile_pool(name="ysb", bufs=2) as ysbp,
            ):
                pown_sb = [
                    pwp.tile([128, (s + 1) * 512], BF16, name=f"pown{s}")
                    for s in range(4)
                ]
                nc.sync.dma_start(pown_sb[3][:], p_own[3][:])
                nc.sync.dma_start(pown_sb[2][:], p_own[2][:])
                vf = dp.tile([128, 4, NT, 512], BF16)  # [k-part, g, kt, dv-in-g]
                for g in range(4):
                    q_eng = nc.sync if g % 2 == 0 else nc.scalar
                    q_eng.dma_start(vf[:, g, :, :], v_ag[g][:, :, :])
                wo_sb = dp.tile([128, NT, D], BF16)
                nc.scalar.dma_start(wo_sb[:], wo_r[:])
                pt = dp.tile([128, NT, 512], BF16)  # [k-part, kt, own-q-col]
                ot = dp.tile([128, NT, 512], BF16)  # [dv-part, dvt, own-q]

                with (
                    tc.tile_pool(name="tp_ps", bufs=2, space="PSUM") as tpp,
                    tc.tile_pool(name="ot_ps", bufs=2, space="PSUM") as otbp,
                    tc.tile_pool(name="y_ps", bufs=2, space="PSUM") as yps,
                ):
                    def transpose_slab(s_idx):
                        for kt in range(4 * s_idx + 4):
                            ps = tpp.tile([128, 128], BF16, tag="tp")
                            nc.tensor.transpose(
                                ps[:],
                                pown_sb[s_idx][:, kt * 128 : (kt + 1) * 128],
                                ident_bf[:],
                            )
                            nc.scalar.copy(
                                out=pt[:, kt, s_idx * 128 : (s_idx + 1) * 128],
                                in_=ps[:],
                            )

                    def ot_pass(base, ktn):
                        # OT for own q-cols [base, base+256): k-tiles 0..ktn-1
                        for dvt in range(NT):
                            po = otbp.tile([128, 256], F32, tag="OTB")
                            for kt in range(ktn):
                                c0 = max(0, 128 * (kt // 4) - base)
                                nc.tensor.matmul(
                                    po[:, c0:256],
                                    vf[
                                        :,
                                        dvt // 4,
                                        kt,
                                        (dvt % 4) * 128 : (dvt % 4) * 128 + 128,
                                    ],
                                    pt[:, kt, base + c0 : base + 256],
                                    start=(kt == 0),
                                    stop=(kt == ktn - 1),
                                )
                            nc.vector.tensor_copy(
                                out=ot[:, dvt, base : base + 256], in_=po[:]
                            )

                    def y_pass(qb):
                        for nch in range(4):
                            yp = yps.tile([128, 512], F32, tag="Y")
                            for dvt in range(NT):
                                nc.tensor.matmul(
                                    yp[:],
                                    ot[:, dvt, qb * 128 : (qb + 1) * 128],
                                    wo_sb[:, dvt, nch * 512 : nch * 512 + 512],
                                    start=(dvt == 0),
                                    stop=(dvt == NT - 1),
                                )
                            y_sb = ysbp.tile([128, 512], F32, tag="ysb")
                            nc.scalar.copy(out=y_sb[:], in_=yp[:])
                            nc.sync.dma_start(
                                out[
                                    qb * 128 : (qb + 1) * 128,
                                    nch * 512 : nch * 512 + 512,
                                ],
                                y_sb[:],
                            )

                    transpose_slab(3)
                    transpose_slab(2)
                    ot_pass(256, 16)
                    y_pass(3)
                    y_pass(2)
                    nc.sync.dma_start(pown_sb[1][:], p_own[1][:])
                    nc.sync.dma_start(pown_sb[0][:], p_own[0][:])
                    transpose_slab(1)
                    transpose_slab(0)
                    ot_pass(0, 8)
                    y_pass(1)
                    y_pass(0)

    nc.compile()
    return nc


_NC_CACHE = None


def kernel(x, W_q, W_k, W_v, W_o):
    global _NC_CACHE
    x = np.asarray(x, dtype=np.float32)
    W_q = np.asarray(W_q, dtype=np.float32)
    W_k = np.asarray(W_k, dtype=np.float32)
    W_v = np.asarray(W_v, dtype=np.float32)
    W_o = np.asarray(W_o, dtype=np.float32)
    if _NC_CACHE is None:
        _NC_CACHE = build()
    nc = _NC_CACHE

    wo_bf = W_o.astype(ml_dtypes.bfloat16)
    xT = [np.ascontiguousarray(x[g].T) for g in range(2)]
    xT_bf = [t.astype(ml_dtypes.bfloat16) for t in xT]
    in_maps = []
    for c in range(8):
        g, r = divmod(c, 4)
        in_maps.append(
            {
                "x": xT[g],
                "xbf": xT_bf[g],
                "wq": np.ascontiguousarray(W_q[:, 512 * r : 512 * (r + 1)]),
                "wk": np.ascontiguousarray(W_k[:, 512 * r : 512 * (r + 1)]),
                "wv": np.ascontiguousarray(W_v[:, 512 * r : 512 * (r + 1)]).astype(ml_dtypes.bfloat16),
                "wo": wo_bf,
            }
        )
    res = run_bass_kernel_spmd(nc, in_maps, core_ids=list(range(8)))
    Y = np.empty((2, S, D), dtype=np.float32)
    for c in range(8):
        g, r = divmod(c, 4)
        o = res.results[c]["out"]
        for s_idx in range(4):
            t = 4 * s_idx + r
            Y[g, t * 128 : (t + 1) * 128, :] = o[s_idx * 128 : (s_idx + 1) * 128, :]
    return Y
